# revision 1
# baseline (speedup 1.0000x reference)
"""Multi-head attention Bass kernel for Trainium2, sharded over 8 NeuronCores.

Sharding: core c handles batch b = c//4 and head-group g = c%4 (4 of 16 heads,
i.e. a 256-wide slice of the QKV projection output).  Each core computes its
heads' attention and a partial output projection (contribution of its 256
ctx columns to the full [S, D] output).  The host sums the 4 partials per
batch and adds the output bias.

Device-side layout choices:
  - activations shipped pre-transposed: xT = x.T  [D, S] so the contraction
    dim (D) lands on SBUF partitions without any on-device transpose.
  - scores are computed transposed (scoresT[sk, sq]) so the attention weights
    leave softmax with sk on partitions — the contraction layout attn@V needs.
  - softmax denominator comes free from a ones-column appended to V
    (ctx psum row 64 = sum_sk attn);  no max-subtraction (scores bounded).
  - masking is a multiply by a 0/1 bf16 keep-mask after exp.
  - projections run kk-outer (one xT tile DMA feeds one matmul burst across
    all open psum groups) so the PE stream is dense and the HAM clock-gate
    un-throttles; stage B processes heads in opposite-parity pairs so the
    K=64 score matmuls row-tile into concurrent halves of the PE array.
"""

import numpy as np
import ml_dtypes

import concourse.bass as bass
import concourse.mybir as mybir
import concourse.tile as tile
from concourse import bacc, library_config
from concourse.bass_utils import run_bass_kernel_spmd

# Problem shapes (hardcoded per contest rules).
B, S, D, H, DH = 2, 2048, 1024, 16, 64
NCORES = 8
NH = 4            # heads per core
DQ = NH * DH      # 256: per-core q/k/v width
P = 128

F32 = mybir.dt.float32
BF16 = mybir.dt.bfloat16
NP_BF16 = ml_dtypes.bfloat16

# Tunables.
SQ_CHUNK = 1024   # sq block processed per inner pipeline (psum-bank limited)
FDP = 512         # matmul moving free-dim (one fp32 psum bank)
ZR = 512          # psum zero-region, fp32 elements (accumulation-group grain)


def build_nc(s=S, d=D, sq_chunk=SQ_CHUNK):
    """Build the per-core Bass program (same NEFF on all 8 cores)."""
    ko = d // P           # contraction chunks for projections
    mq = DQ // P          # 2: q/k partition chunks
    sk_n = s // P         # sk chunks
    nsq = s // sq_chunk
    fdp = min(FDP, sq_chunk)
    nj = sq_chunk // fdp  # matmuls per score slab
    pss = max(sq_chunk, DQ)  # psum tile free size
    gpt = max(1, pss // ZR)  # independent accum groups per psum tile

    nc = bacc.Bacc("TRN2", debug=False)

    xq_t = nc.declare_dram_parameter("xq", [d, s], BF16, isOutput=False)
    xk_t = nc.declare_dram_parameter("xk", [d, s], BF16, isOutput=False)
    xv_t = nc.declare_dram_parameter("xv", [d, s], BF16, isOutput=False)
    wq_t = nc.declare_dram_parameter("wq", [d, DQ], BF16, isOutput=False)
    wk_t = nc.declare_dram_parameter("wk", [d, DQ], BF16, isOutput=False)
    wv_t = nc.declare_dram_parameter("wv", [d, DQ], BF16, isOutput=False)
    wo_t = nc.declare_dram_parameter("wo", [DQ, d], BF16, isOutput=False)
    bq_t = nc.declare_dram_parameter("bq", [P, mq], F32, isOutput=False)
    bk_t = nc.declare_dram_parameter("bk", [P, mq], F32, isOutput=False)
    bv_t = nc.declare_dram_parameter("bv", [P, DQ], F32, isOutput=False)
    keep_t = nc.declare_dram_parameter("keep", [s, s], BF16, isOutput=False)
    out_t = nc.declare_dram_parameter("out", [d, s], F32, isOutput=True)

    AF = mybir.ActivationFunctionType
    OP = mybir.AluOpType

    with tile.TileContext(nc) as tc:
        nc.gpsimd.load_library(library_config.attn)
        with (
            tc.tile_pool(name="const", bufs=1) as const,
            tc.tile_pool(name="xs", bufs=4) as xs,
            tc.tile_pool(name="attn", bufs=4) as attnp,
            tc.tile_pool(name="sc", bufs=2) as scp,
            tc.tile_pool(name="outp", bufs=3) as outp,
            tc.tile_pool(name="ps", bufs=2, space="PSUM") as psp,
            tc.tile_pool(name="psc", bufs=2, space="PSUM") as psc,
        ):
            # ---- persistent SBUF tensors ----
            wq_sb = const.tile([P, ko, DQ], BF16, tag="wq")
            wk_sb = const.tile([P, ko, DQ], BF16, tag="wk")
            wv_sb = const.tile([P, ko, DQ], BF16, tag="wv")
            wo_sb = const.tile([P, mq, d], BF16, tag="wo")
            bq_sb = const.tile([P, mq], F32, tag="bq")
            bk_sb = const.tile([P, mq], F32, tag="bk")
            bv_sb = const.tile([P, DQ], F32, tag="bv")
            qT_sb = const.tile([P, mq, s], BF16, tag="qT")
            kT_sb = const.tile([P, mq, s], BF16, tag="kT")
            v_sb = const.tile([P, sk_n, NH * 65], BF16, tag="v")
            keep_sb = const.tile([P, sk_n, s], BF16, tag="keep")
            ctxT_sb = const.tile([P, mq, s], BF16, tag="ctxT")

            nc.sync.dma_start(wq_sb, wq_t[:].rearrange("(ko p) m -> p ko m", p=P))
            nc.sync.dma_start(wk_sb, wk_t[:].rearrange("(ko p) m -> p ko m", p=P))
            nc.sync.dma_start(wv_sb, wv_t[:].rearrange("(ko p) m -> p ko m", p=P))
            nc.sync.dma_start(wo_sb, wo_t[:].rearrange("(mq p) n -> p mq n", p=P))
            nc.sync.dma_start(bq_sb, bq_t[:])
            nc.sync.dma_start(bk_sb, bk_t[:])
            nc.sync.dma_start(bv_sb, bv_t[:])

            # ones column per head in the V tile (softmax denominator trick)
            nc.vector.memset(
                v_sb[:].rearrange("p s (h c) -> p s h c", h=NH)[:, :, :, 64:65], 1.0
            )

            def alloc_group_tiles(n_groups):
                """Allocate psum tiles hosting `n_groups` independent
                accumulation groups (one per zero-region slot)."""
                tiles = []
                need = (n_groups + gpt - 1) // gpt
                for i in range(need):
                    pool = psp if i % 2 == 0 else psc
                    gt = pool.tile([P, pss], F32, name=f"gt{i}",
                                   tag="s" if i % 2 == 0 else "c")
                    tiles.append(gt)
                return tiles

            def gslice(tiles, g, width):
                return tiles[g // gpt][:, (g % gpt) * ZR:(g % gpt) * ZR + width]

            # ---- stage A: projections, kk-outer (dense PE stream) ----
            nsf = s // fdp

            def project_qk(x_t, w_sb, b_sb, dst_sb):
                ngr = mq * nsf
                tiles = alloc_group_tiles(ngr)
                for kk in range(ko):
                    t = xs.tile([P, s], BF16, tag="xt")
                    nc.sync.dma_start(t, x_t[kk * P:(kk + 1) * P, :])
                    for m in range(mq):
                        for n in range(nsf):
                            g = m * nsf + n
                            nc.tensor.matmul(
                                gslice(tiles, g, fdp),
                                w_sb[:, kk, m * P:(m + 1) * P],
                                t[:, n * fdp:(n + 1) * fdp],
                                start=(kk == 0),
                                stop=(kk == ko - 1),
                            )
                for m in range(mq):
                    for n in range(nsf):
                        g = m * nsf + n
                        nc.vector.tensor_scalar_add(
                            dst_sb[:, m, n * fdp:(n + 1) * fdp],
                            gslice(tiles, g, fdp),
                            b_sb[:, m:m + 1],
                        )

            project_qk(xq_t, wq_sb, bq_sb, qT_sb)
            project_qk(xk_t, wk_sb, bk_sb, kT_sb)

            # v projection: v[sv, dv] = sum_d xvT[d, sv] * wvT[d, dv]
            # kk-outer in waves of up to 8 sv-chunks (re-DMAs xvT per wave)
            v_strided = v_sb[:].rearrange("p s (h c) -> p s h c", h=NH)
            wave = min(sk_n, 4 * gpt)
            for w0 in range(0, sk_n, wave):
                nsv = min(wave, sk_n - w0)
                tiles = alloc_group_tiles(nsv)
                for kk in range(ko):
                    t = xs.tile([P, s], BF16, tag="xt")
                    nc.sync.dma_start(t, xv_t[kk * P:(kk + 1) * P, :])
                    for g in range(nsv):
                        sv = w0 + g
                        nc.tensor.matmul(
                            gslice(tiles, g, DQ),
                            t[:, sv * P:(sv + 1) * P],
                            wv_sb[:, kk, :],
                            start=(kk == 0),
                            stop=(kk == ko - 1),
                        )
                for g in range(nsv):
                    sv = w0 + g
                    nc.vector.tensor_tensor(
                        v_strided[:, sv, :, 0:64],
                        gslice(tiles, g, DQ).rearrange("p (h c) -> p h c", h=NH),
                        bv_sb[:].rearrange("p (h c) -> p h c", h=NH),
                        OP.add,
                    )

            # keep-mask: [sk partitions, sq free]
            for c in range(sk_n):
                nc.sync.dma_start(keep_sb[:, c, :], keep_t[c * P:(c + 1) * P, :])

            def normalize(cps, h, sq0):
                """ctx[0:64] /= den[64]; write into ctxT_sb (repacked).
                HW quirk: custom-DVE / gpsimd ops only work at base partition
                0, so the den row is copied out of psum (standard DVE op, base
                64 OK) and shifted to partition 0 via an SBUF-SBUF DMA."""
                hb, hm = (h % 2) * 64, h // 2
                den = scp.tile([65, sq_chunk], F32, tag="den")
                nc.vector.tensor_copy(den[64:65, :], cps[64:65, :])
                den0 = scp.tile([1, sq_chunk], F32, tag="den0")
                nc.sync.dma_start(den0, den[64:65, :])
                nc.vector.reciprocal_approx_fast(out=den0, in_=den0)
                scl = scp.tile([64, sq_chunk], F32, tag="scl")
                nc.gpsimd.partition_broadcast(scl, den0[0:1, :])
                cn = scp.tile([64, sq_chunk], BF16, tag="cn")
                nc.vector.tensor_tensor(cn, cps[0:64, :], scl, OP.mult)
                nc.sync.dma_start(ctxT_sb[hb:hb + 64, hm, sq0:sq0 + sq_chunk], cn)

            # ---- stage B: attention, opposite-parity head pairs ----
            for sqh in range(nsq):
                sq0 = sqh * sq_chunk
                for pair in range(NH // 2):
                    hs = (2 * pair, 2 * pair + 1)   # parities 0 and 1
                    cpss = [psc.tile([P, pss], F32, name=f"cps{i}",
                                     tag="c")[:65, :sq_chunk]
                            for i in range(2)]
                    for sk in range(sk_n):
                        spss = [psp.tile([P, pss], F32, name=f"sps{i}",
                                         tag="s")[:, :sq_chunk]
                                for i in range(2)]
                        for j in range(nj):
                            for i, h in enumerate(hs):
                                hb, hm = (h % 2) * 64, h // 2
                                nc.tensor.matmul(
                                    spss[i][:, j * fdp:(j + 1) * fdp],
                                    kT_sb[hb:hb + 64, hm, sk * P:(sk + 1) * P],
                                    qT_sb[hb:hb + 64, hm,
                                          sq0 + j * fdp:sq0 + (j + 1) * fdp],
                                    start=True,
                                    stop=True,
                                )
                        ats = []
                        for i, h in enumerate(hs):
                            at = attnp.tile([P, sq_chunk], BF16, tag="at")
                            nc.scalar.activation(at, spss[i], AF.Exp, scale=0.125)
                            nc.vector.tensor_tensor(
                                at, at, keep_sb[:, sk, sq0:sq0 + sq_chunk],
                                OP.mult,
                            )
                            ats.append(at)
                        for i, h in enumerate(hs):
                            for j in range(nj):
                                nc.tensor.matmul(
                                    cpss[i][:, j * fdp:(j + 1) * fdp],
                                    v_sb[:, sk, h * 65:(h + 1) * 65],
                                    ats[i][:, j * fdp:(j + 1) * fdp],
                                    start=(sk == 0),
                                    stop=(sk == sk_n - 1),
                                )
                    for i, h in enumerate(hs):
                        normalize(cpss[i], h, sq0)

                # output projection for this sq block
                for do in range(ko):
                    for nn in range(sq_chunk // fdp):
                        ps = psp.tile([P, pss], F32, tag="s")
                        for kk in range(mq):
                            nc.tensor.matmul(
                                ps[:, :fdp],
                                wo_sb[:, kk, do * P:(do + 1) * P],
                                ctxT_sb[:, kk,
                                        sq0 + nn * fdp:sq0 + (nn + 1) * fdp],
                                start=(kk == 0),
                                stop=(kk == mq - 1),
                            )
                        ot = outp.tile([P, fdp], F32, tag="ot")
                        nc.vector.tensor_copy(ot, ps[:, :fdp])
                        nc.sync.dma_start(
                            out_t[do * P:(do + 1) * P,
                                  sq0 + nn * fdp:sq0 + (nn + 1) * fdp],
                            ot,
                        )
    nc.compile()
    return nc


_NC_CACHE = {}


def _get_nc(s=S, d=D):
    key = (s, d, SQ_CHUNK)
    if key not in _NC_CACHE:
        _NC_CACHE[key] = build_nc(s, d)
    return _NC_CACHE[key]


def make_in_maps(query, key, value, mask, Wq, bq, Wk, bk, Wv, bv, Wo, bo,
                 s=S, d=D):
    """Build the 8 per-core input maps (host-side shard + layout prep)."""
    nb = query.shape[0]
    per_b = []
    for b in range(nb):
        xqT = np.ascontiguousarray(query[b].T).astype(NP_BF16)
        xkT = np.ascontiguousarray(key[b].T).astype(NP_BF16)
        xvT = np.ascontiguousarray(value[b].T).astype(NP_BF16)
        keepT = np.ascontiguousarray((~mask[b, 0]).T).astype(NP_BF16)
        per_b.append((xqT, xkT, xvT, keepT))
    per_g = []
    for g in range(4):
        sl = slice(g * DQ, (g + 1) * DQ)
        per_g.append((
            np.ascontiguousarray(Wq[sl].T).astype(NP_BF16),
            np.ascontiguousarray(Wk[sl].T).astype(NP_BF16),
            np.ascontiguousarray(Wv[sl].T).astype(NP_BF16),
            np.ascontiguousarray(Wo[:, sl].T).astype(NP_BF16),
            np.ascontiguousarray(bq[sl].reshape(DQ // P, P).T).astype(np.float32),
            np.ascontiguousarray(bk[sl].reshape(DQ // P, P).T).astype(np.float32),
            np.ascontiguousarray(np.broadcast_to(bv[sl], (P, DQ))).astype(np.float32),
        ))
    in_maps = []
    for c in range(NCORES):
        b, g = c // 4, c % 4
        xqT, xkT, xvT, keepT = per_b[b % nb]
        wqT, wkT, wvT, woT, bq2, bk2, bvr = per_g[g]
        in_maps.append({
            "xq": xqT, "xk": xkT, "xv": xvT,
            "wq": wqT, "wk": wkT, "wv": wvT, "wo": woT,
            "bq": bq2, "bk": bk2, "bv": bvr,
            "keep": keepT,
        })
    return in_maps


def gather_output(results, bo, nb=B, s=S, d=D):
    out = np.empty((nb, s, d), np.float32)
    for b in range(nb):
        acc = results[4 * b]["out"].copy()
        for g in range(1, 4):
            acc += results[4 * b + g]["out"]
        out[b] = acc.T
    out += bo.astype(np.float32)
    return out


def run_on_cores(in_maps, trace=False, **kw):
    nc = _get_nc()
    return run_bass_kernel_spmd(nc, in_maps, list(range(NCORES)), trace=trace, **kw)


def kernel(query, key, value, mask, Wq, bq, Wk, bk, Wv, bv, Wo, bo):
    in_maps = make_in_maps(query, key, value, mask,
                           Wq, bq, Wk, bk, Wv, bv, Wo, bo)
    res = run_on_cores(in_maps, trace=False)
    return gather_output(res.results, bo)



# revision 2
# speedup vs baseline: 1.2287x; 1.2287x over previous
"""Multi-head attention Bass kernel for Trainium2, sharded over 8 NeuronCores.

Sharding: core c handles batch b = c//4 and head-group g = c%4 (4 of 16 heads,
i.e. a 256-wide slice of the QKV projection output).  Each core computes its
heads' attention and a partial output projection (contribution of its 256
ctx columns to the full [S, D] output).  The host sums the 4 partials per
batch (fp32) and adds the output bias.

Device-side design (v2 — ScalarE-saturated pipeline):
  - activations shipped pre-transposed: xT = x.T  [D, S] so the contraction
    dim (D) lands on SBUF partitions without any on-device transpose.
  - scores are computed transposed (scoresT[sk, sq]) so the attention weights
    leave softmax with sk on partitions — the contraction layout attn@V needs.
  - softmax denominator comes free from a ones-column appended to V
    (ctx psum row 64 = sum_sk attn);  no max-subtraction (scores bounded).
  - stage B processes one head at a time, sweeping sk chunks with a
    double-buffered score psum: the PE issues scores(sk+2) the moment
    exp(sk) frees a buffer, so the Exp stream on ScalarE (the critical
    engine: ~1 elem/cycle/lane) never waits and the PE never idles long
    enough for the HAM clock-gate to re-throttle.
  - output projection is emitted one unit per sweep step into the PE's
    idle slack during the NEXT sq block's sweeps (psum: 2 dedicated banks).
  - psum budget (16KB/part): score 2x[128,1024]f32 + ctx 1x[128,1024]f32
    + oproj 2x[128,512]f32 = 8 banks exactly.  Stage A reuses all four
    pools as 8 independent 512-wide accumulation slots, kk-outer.
"""

import numpy as np
import ml_dtypes

import concourse.bass as bass
import concourse.mybir as mybir
import concourse.tile as tile
from concourse import bacc, library_config
from concourse.bass_utils import run_bass_kernel_spmd

# Problem shapes (hardcoded per contest rules).
B, S, D, H, DH = 2, 2048, 1024, 16, 64
NCORES = 8
NH = 4            # heads per core
DQ = NH * DH      # 256: per-core q/k/v width
P = 128

F32 = mybir.dt.float32
F16 = mybir.dt.float16
BF16 = mybir.dt.bfloat16
NP_BF16 = ml_dtypes.bfloat16

SQC = 1024        # sq block per sweep
NSQ = S // SQC    # 2
SKN = S // P      # 16 sk chunks
KO = D // P       # 8 contraction chunks for projections
MQ = DQ // P      # 2


def build_nc():
    """Build the per-core Bass program (same NEFF on all 8 cores)."""
    nc = bacc.Bacc("TRN2", debug=False)

    xq_t = nc.declare_dram_parameter("xq", [D, S], BF16, isOutput=False)
    xk_t = nc.declare_dram_parameter("xk", [D, S], BF16, isOutput=False)
    xv_t = nc.declare_dram_parameter("xv", [D, S], BF16, isOutput=False)
    wq_t = nc.declare_dram_parameter("wq", [D, DQ], BF16, isOutput=False)
    wk_t = nc.declare_dram_parameter("wk", [D, DQ], BF16, isOutput=False)
    wv_t = nc.declare_dram_parameter("wv", [D, DQ], BF16, isOutput=False)
    wo_t = nc.declare_dram_parameter("wo", [DQ, D], BF16, isOutput=False)
    bq_t = nc.declare_dram_parameter("bq", [P, MQ], F32, isOutput=False)
    bk_t = nc.declare_dram_parameter("bk", [P, MQ], F32, isOutput=False)
    bv_t = nc.declare_dram_parameter("bv", [P, DQ], F32, isOutput=False)
    keep_t = nc.declare_dram_parameter("keep", [S, S], BF16, isOutput=False)
    out_t = nc.declare_dram_parameter("out", [D, S], F16, isOutput=True)

    AF = mybir.ActivationFunctionType
    OP = mybir.AluOpType

    with tile.TileContext(nc) as tc:
        nc.gpsimd.load_library(library_config.attn)
        with (
            tc.tile_pool(name="const", bufs=1) as const,
            tc.tile_pool(name="xs", bufs=3) as xs,
            tc.tile_pool(name="attn", bufs=6) as attnp,
            tc.tile_pool(name="sc", bufs=2) as scp,
            tc.tile_pool(name="outp", bufs=3) as outp,
            tc.tile_pool(name="ps_s", bufs=2, space="PSUM") as ps_s,
            tc.tile_pool(name="ps_c", bufs=1, space="PSUM") as ps_c,
            tc.tile_pool(name="ps_o", bufs=2, space="PSUM") as ps_o,
        ):
            # ---- persistent SBUF tensors ----
            wq_sb = const.tile([P, KO, DQ], BF16, tag="wq")
            wk_sb = const.tile([P, KO, DQ], BF16, tag="wk")
            wv_sb = const.tile([P, KO, DQ], BF16, tag="wv")
            wo_sb = const.tile([P, MQ, D], BF16, tag="wo")
            bq_sb = const.tile([P, MQ], F32, tag="bq")
            bk_sb = const.tile([P, MQ], F32, tag="bk")
            bv_sb = const.tile([P, DQ], F32, tag="bv")
            qT_sb = const.tile([P, MQ, S], BF16, tag="qT")
            kT_sb = const.tile([P, MQ, S], BF16, tag="kT")
            v_sb = const.tile([P, SKN, NH * 65], BF16, tag="v")
            keep_sb = const.tile([P, SKN, S], BF16, tag="keep")
            ctxT_sb = const.tile([P, MQ, S], BF16, tag="ctxT")
            xv_sb = const.tile([P, KO, S], BF16, tag="xv")
            warm = const.tile([1, 8], F32, tag="warm")

            # preload the exp table set on ScalarE while stage A runs
            nc.vector.memset(warm, 0.0)
            nc.scalar.activation(warm, warm, AF.Exp, scale=1.0)

            # ones column per head in the V tile (softmax denominator trick)
            v_strided = v_sb[:].rearrange("p s (h c) -> p s h c", h=NH)
            nc.vector.memset(v_strided[:, :, :, 64:65], 1.0)

            def psum_slots():
                """8 independent [128,512] accumulation slots spanning all
                psum pools (stage A only; stage B owns the pools then)."""
                t0 = ps_s.tile([P, SQC], F32, tag="s")
                t1 = ps_s.tile([P, SQC], F32, tag="s")
                tcx = ps_c.tile([P, SQC], F32, tag="c")
                o0 = ps_o.tile([P, 512], F32, tag="o")
                o1 = ps_o.tile([P, 512], F32, tag="o")
                return [t0[:, 0:512], t0[:, 512:1024],
                        t1[:, 0:512], t1[:, 512:1024],
                        tcx[:, 0:512], tcx[:, 512:1024],
                        o0[:, :], o1[:, :]]

            # ---- stage A: projections, kk-outer (dense PE stream) ----
            def proj_qk(x_t, w_sb, b_sb, dst_sb, extra_dma=None):
                slots = psum_slots()
                for kk in range(KO):
                    t = xs.tile([P, S], BF16, tag="xt")
                    nc.sync.dma_start(t, x_t[kk * P:(kk + 1) * P, :])
                    if extra_dma is not None:
                        extra_dma(kk)
                    for g in range(8):
                        m, n = g // 4, g % 4
                        nc.tensor.matmul(
                            slots[g],
                            w_sb[:, kk, m * P:(m + 1) * P],
                            t[:, n * 512:(n + 1) * 512],
                            start=(kk == 0),
                            stop=(kk == KO - 1),
                        )
                        if kk == KO - 1:
                            # bias-add right behind each group's stop-MM so
                            # psum frees with no end-of-projection bubble
                            nc.vector.tensor_scalar_add(
                                dst_sb[:, m, n * 512:(n + 1) * 512],
                                slots[g], b_sb[:, m:m + 1],
                            )

            def proj_v():
                # v[sv, dv] = sum_d xvT[d, sv] * wvT[d, dv], from resident xv
                for w0 in (0, 8):
                    slots = psum_slots()
                    for kk in range(KO):
                        for g in range(8):
                            sv = w0 + g
                            nc.tensor.matmul(
                                slots[g][:, 0:DQ],
                                xv_sb[:, kk, sv * P:(sv + 1) * P],
                                wv_sb[:, kk, :],
                                start=(kk == 0),
                                stop=(kk == KO - 1),
                            )
                            if kk == KO - 1:
                                nc.vector.tensor_tensor(
                                    v_strided[:, sv, :, 0:64],
                                    slots[g][:, 0:DQ].rearrange(
                                        "p (h c) -> p h c", h=NH),
                                    bv_sb[:].rearrange("p (h c) -> p h c", h=NH),
                                    OP.add,
                                )

            nc.sync.dma_start(wk_sb, wk_t[:].rearrange("(ko p) m -> p ko m", p=P))
            nc.sync.dma_start(bk_sb, bk_t[:])
            proj_qk(xk_t, wk_sb, bk_sb, kT_sb)

            nc.sync.dma_start(wq_sb, wq_t[:].rearrange("(ko p) m -> p ko m", p=P))
            nc.sync.dma_start(bq_sb, bq_t[:])
            nc.sync.dma_start(wv_sb, wv_t[:].rearrange("(ko p) m -> p ko m", p=P))
            nc.sync.dma_start(bv_sb, bv_t[:])
            # interleave xv prefetch with Q's just-in-time x tiles
            proj_qk(xq_t, wq_sb, bq_sb, qT_sb,
                    extra_dma=lambda kk: nc.sync.dma_start(
                        xv_sb[:, kk, :], xv_t[kk * P:(kk + 1) * P, :]))
            proj_v()

            # mask chunks, in stage-B consumption order (sqh-major)
            for sqh in range(NSQ):
                for c in range(SKN):
                    nc.sync.dma_start(
                        keep_sb[:, c, sqh * SQC:(sqh + 1) * SQC],
                        keep_t[c * P:(c + 1) * P, sqh * SQC:(sqh + 1) * SQC])
            nc.sync.dma_start(wo_sb, wo_t[:].rearrange("(mq p) n -> p mq n", p=P))

            # ---- stage B: attention, ScalarE-saturated per-head sweeps ----
            def normalize(cps, h, sq0):
                """ctx[0:64] /= den[64]; write into ctxT_sb (repacked).
                HW quirk: custom-DVE / gpsimd ops only work at base partition
                0, so the den row is copied out of psum (standard DVE op, base
                64 OK) and shifted to partition 0 via an SBUF-SBUF DMA."""
                hb, hm = (h % 2) * 64, h // 2
                den = scp.tile([65, SQC], F32, tag="den")
                nc.vector.tensor_copy(den[64:65, :], cps[64:65, :])
                den0 = scp.tile([1, SQC], F32, tag="den0")
                nc.sync.dma_start(den0, den[64:65, :])
                nc.vector.reciprocal_approx_fast(out=den0, in_=den0)
                scl = scp.tile([64, SQC], F32, tag="scl")
                nc.gpsimd.partition_broadcast(scl, den0[0:1, :])
                cn = scp.tile([64, SQC], BF16, tag="cn")
                nc.vector.tensor_tensor(cn, cps[0:64, :], scl, OP.mult)
                nc.sync.dma_start(ctxT_sb[hb:hb + 64, hm, sq0:sq0 + SQC], cn)

            def oproj_unit(do, n2, sq0):
                ps = ps_o.tile([P, 512], F32, tag="o")
                for kk in range(MQ):
                    nc.tensor.matmul(
                        ps,
                        wo_sb[:, kk, do * P:(do + 1) * P],
                        ctxT_sb[:, kk, sq0 + n2 * 512:sq0 + (n2 + 1) * 512],
                        start=(kk == 0),
                        stop=(kk == MQ - 1),
                    )
                ot = outp.tile([P, 512], F16, tag="ot")
                nc.vector.tensor_copy(ot, ps)
                nc.sync.dma_start(
                    out_t[do * P:(do + 1) * P,
                          sq0 + n2 * 512:sq0 + (n2 + 1) * 512], ot)

            def sweep(h, sq0, ounits):
                hb, hm = (h % 2) * 64, h // 2
                cps = ps_c.tile([P, SQC], F32, tag="c")

                def scores(sk):
                    sps = ps_s.tile([P, SQC], F32, tag="s")
                    for j in range(2):
                        nc.tensor.matmul(
                            sps[:, j * 512:(j + 1) * 512],
                            kT_sb[hb:hb + 64, hm, sk * P:(sk + 1) * P],
                            qT_sb[hb:hb + 64, hm,
                                  sq0 + j * 512:sq0 + (j + 1) * 512],
                            start=True, stop=True,
                        )
                    return sps

                pend = [scores(0), scores(1)]
                for sk in range(SKN):
                    sps = pend.pop(0)
                    at = attnp.tile([P, SQC], BF16, tag="at")
                    nc.scalar.activation(at, sps, AF.Exp, scale=0.125)
                    nc.vector.tensor_tensor(
                        at, at, keep_sb[:, sk, sq0:sq0 + SQC], OP.mult)
                    if sk + 2 < SKN:
                        pend.append(scores(sk + 2))
                    if ounits and sk >= 8:
                        oproj_unit(*ounits.pop(0))
                    for j in range(2):
                        nc.tensor.matmul(
                            cps[:65, j * 512:(j + 1) * 512],
                            v_sb[:, sk, h * 65:(h + 1) * 65],
                            at[:, j * 512:(j + 1) * 512],
                            start=(sk == 0),
                            stop=(sk == SKN - 1),
                        )
                normalize(cps, h, sq0)

            ounits = []
            for sqh in range(NSQ):
                sq0 = sqh * SQC
                for h in range(NH):
                    sweep(h, sq0, ounits)
                ounits.extend((do, n2, sq0)
                              for do in range(KO) for n2 in range(2))
            for u in ounits:
                oproj_unit(*u)

    nc.compile()
    return nc


_NC_CACHE = {}


def _get_nc():
    if "nc" not in _NC_CACHE:
        _NC_CACHE["nc"] = build_nc()
    return _NC_CACHE["nc"]


def make_in_maps(query, key, value, mask, Wq, bq, Wk, bk, Wv, bv, Wo, bo):
    """Build the 8 per-core input maps (host-side shard + layout prep)."""
    nb = query.shape[0]
    per_b = []
    for b in range(nb):
        xqT = np.ascontiguousarray(query[b].T).astype(NP_BF16)
        xkT = np.ascontiguousarray(key[b].T).astype(NP_BF16)
        xvT = np.ascontiguousarray(value[b].T).astype(NP_BF16)
        keepT = np.ascontiguousarray((~mask[b, 0]).T).astype(NP_BF16)
        per_b.append((xqT, xkT, xvT, keepT))
    per_g = []
    for g in range(4):
        sl = slice(g * DQ, (g + 1) * DQ)
        per_g.append((
            np.ascontiguousarray(Wq[sl].T).astype(NP_BF16),
            np.ascontiguousarray(Wk[sl].T).astype(NP_BF16),
            np.ascontiguousarray(Wv[sl].T).astype(NP_BF16),
            np.ascontiguousarray(Wo[:, sl].T).astype(NP_BF16),
            np.ascontiguousarray(bq[sl].reshape(DQ // P, P).T).astype(np.float32),
            np.ascontiguousarray(bk[sl].reshape(DQ // P, P).T).astype(np.float32),
            np.ascontiguousarray(np.broadcast_to(bv[sl], (P, DQ))).astype(np.float32),
        ))
    in_maps = []
    for c in range(NCORES):
        b, g = c // 4, c % 4
        xqT, xkT, xvT, keepT = per_b[b % nb]
        wqT, wkT, wvT, woT, bq2, bk2, bvr = per_g[g]
        in_maps.append({
            "xq": xqT, "xk": xkT, "xv": xvT,
            "wq": wqT, "wk": wkT, "wv": wvT, "wo": woT,
            "bq": bq2, "bk": bk2, "bv": bvr,
            "keep": keepT,
        })
    return in_maps


def gather_output(results, bo, nb=B, s=S, d=D):
    out = np.empty((nb, s, d), np.float32)
    for b in range(nb):
        acc = results[4 * b]["out"].astype(np.float32)
        for g in range(1, 4):
            acc += results[4 * b + g]["out"].astype(np.float32)
        out[b] = acc.T
    out += bo.astype(np.float32)
    return out


def run_on_cores(in_maps, trace=False, **kw):
    nc = _get_nc()
    return run_bass_kernel_spmd(nc, in_maps, list(range(NCORES)), trace=trace, **kw)


def kernel(query, key, value, mask, Wq, bq, Wk, bk, Wv, bv, Wo, bo):
    in_maps = make_in_maps(query, key, value, mask,
                           Wq, bq, Wk, bk, Wv, bv, Wo, bo)
    res = run_on_cores(in_maps, trace=False)
    return gather_output(res.results, bo)


# revision 3
# speedup vs baseline: 1.4027x; 1.1416x over previous
"""Multi-head attention Bass kernel for Trainium2, sharded over 8 NeuronCores.

Sharding: core c handles batch b = c//4 and head-group g = c%4 (4 of 16 heads,
i.e. a 256-wide slice of the QKV projection output).  Each core computes its
heads' attention and a partial output projection (contribution of its 256
ctx columns to the full [S, D] output).  The host sums the 4 partials per
batch (fp32) and adds the output bias.

Device-side design (v3 — ScalarE-saturated pipeline):
  - activations shipped pre-transposed: xT = x.T  [D, S] so the contraction
    dim (D) lands on SBUF partitions without any on-device transpose.
    Weights shipped pre-packed in the exact SBUF layout (contiguous DMA).
  - scores are computed transposed (scoresT[sk, sq]) so the attention weights
    leave softmax with sk on partitions — the contraction layout attn@V needs.
  - softmax denominator comes free from a ones-column appended to V
    (ctx psum row 64 = sum_sk attn);  no max-subtraction (scores bounded).
  - stage B processes one head at a time, sweeping sk chunks with a
    double-buffered score psum: the PE issues scores(sk+2) the moment
    exp(sk) frees a buffer, so the Exp stream on ScalarE (the critical
    engine: ~1 elem/cycle/lane) never waits and the PE never idles long
    enough for the HAM clock-gate to re-throttle.  ctx matmuls trail one
    step behind so psum hand-offs can't head-of-line-block the PE queue.
  - normalize does one staging copy psum->SBUF (fast ctx-psum free for the
    next head) and runs recip/broadcast/divide lazily off the critical path.
  - output projection is emitted one unit per sweep step into the PE's
    idle slack during the NEXT sq block's sweeps; the final block drains
    in a short tail with psum->SBUF copies split across Vector/Scalar.
  - psum budget (16KB/part): score 2x[128,1024]f32 + ctx 1x[128,1024]f32
    + oproj 2x[128,512]f32 = 8 banks exactly.  Stage A reuses all four
    pools as 8 independent 512-wide accumulation slots, kk-outer.
"""

import numpy as np
import ml_dtypes

import concourse.bass as bass
import concourse.mybir as mybir
import concourse.tile as tile
from concourse import bacc, library_config
from concourse.bass_utils import run_bass_kernel_spmd

# Problem shapes (hardcoded per contest rules).
B, S, D, H, DH = 2, 2048, 1024, 16, 64
NCORES = 8
NH = 4            # heads per core
DQ = NH * DH      # 256: per-core q/k/v width
P = 128

F32 = mybir.dt.float32
F16 = mybir.dt.float16
BF16 = mybir.dt.bfloat16
NP_BF16 = ml_dtypes.bfloat16

SQC = 1024        # sq block per sweep
NSQ = S // SQC    # 2
SKN = S // P      # 16 sk chunks
KO = D // P       # 8 contraction chunks for projections
MQ = DQ // P      # 2


def build_nc():
    """Build the per-core Bass program (same NEFF on all 8 cores)."""
    nc = bacc.Bacc("TRN2", debug=False)

    xq_t = nc.declare_dram_parameter("xq", [D, S], BF16, isOutput=False)
    xk_t = nc.declare_dram_parameter("xk", [D, S], BF16, isOutput=False)
    xv_t = nc.declare_dram_parameter("xv", [D, S], BF16, isOutput=False)
    wq_t = nc.declare_dram_parameter("wq", [P, KO * DQ], BF16, isOutput=False)
    wk_t = nc.declare_dram_parameter("wk", [P, KO * DQ], BF16, isOutput=False)
    wv_t = nc.declare_dram_parameter("wv", [P, KO * DQ], BF16, isOutput=False)
    wo_t = nc.declare_dram_parameter("wo", [P, MQ * D], BF16, isOutput=False)
    bq_t = nc.declare_dram_parameter("bq", [P, MQ], F32, isOutput=False)
    bk_t = nc.declare_dram_parameter("bk", [P, MQ], F32, isOutput=False)
    bv_t = nc.declare_dram_parameter("bv", [P, DQ], F32, isOutput=False)
    keep_t = nc.declare_dram_parameter("keep", [S, S], BF16, isOutput=False)
    out_t = nc.declare_dram_parameter("out", [D, S], F16, isOutput=True)

    AF = mybir.ActivationFunctionType
    OP = mybir.AluOpType

    with tile.TileContext(nc) as tc:
        nc.gpsimd.load_library(library_config.attn)
        with (
            tc.tile_pool(name="const", bufs=1) as const,
            tc.tile_pool(name="xs", bufs=3) as xs,
            tc.tile_pool(name="attn", bufs=7) as attnp,
            tc.tile_pool(name="sc", bufs=2) as scp,
            tc.tile_pool(name="outp", bufs=3) as outp,
            tc.tile_pool(name="ps_s", bufs=2, space="PSUM") as ps_s,
            tc.tile_pool(name="ps_c", bufs=1, space="PSUM") as ps_c,
            tc.tile_pool(name="ps_o", bufs=2, space="PSUM") as ps_o,
        ):
            # ---- persistent SBUF tensors ----
            wq_sb = const.tile([P, KO, DQ], BF16, tag="wq")
            wk_sb = const.tile([P, KO, DQ], BF16, tag="wk")
            wv_sb = const.tile([P, KO, DQ], BF16, tag="wv")
            wo_sb = const.tile([P, MQ, D], BF16, tag="wo")
            bq_sb = const.tile([P, MQ], F32, tag="bq")
            bk_sb = const.tile([P, MQ], F32, tag="bk")
            bv_sb = const.tile([P, DQ], F32, tag="bv")
            qT_sb = const.tile([P, MQ, S], BF16, tag="qT")
            kT_sb = const.tile([P, MQ, S], BF16, tag="kT")
            v_sb = const.tile([P, SKN, NH * 65], BF16, tag="v")
            keep_sb = const.tile([P, SKN, S], BF16, tag="keep")
            ctxT_sb = const.tile([P, MQ, S], BF16, tag="ctxT")
            xv_sb = const.tile([P, KO, S], BF16, tag="xv")
            warm = const.tile([1, 8], F32, tag="warm")

            # preload the exp table set on ScalarE while stage A runs
            nc.vector.memset(warm, 0.0)
            nc.scalar.activation(warm, warm, AF.Exp, scale=1.0)

            # ones column per head in the V tile (softmax denominator trick)
            v_strided = v_sb[:].rearrange("p s (h c) -> p s h c", h=NH)
            nc.vector.memset(v_strided[:, :, :, 64:65], 1.0)

            def psum_slots():
                """8 independent [128,512] accumulation slots spanning all
                psum pools (stage A only; stage B owns the pools then)."""
                t0 = ps_s.tile([P, SQC], F32, tag="s")
                t1 = ps_s.tile([P, SQC], F32, tag="s")
                tcx = ps_c.tile([P, SQC], F32, tag="c")
                o0 = ps_o.tile([P, 512], F32, tag="o")
                o1 = ps_o.tile([P, 512], F32, tag="o")
                return [t0[:, 0:512], t0[:, 512:1024],
                        t1[:, 0:512], t1[:, 512:1024],
                        tcx[:, 0:512], tcx[:, 512:1024],
                        o0[:, :], o1[:, :]]

            # ---- stage A: projections, kk-outer (dense PE stream) ----
            def proj_qk(x_t, w_sb, b_sb, dst_sb, extra_dma=None):
                slots = psum_slots()
                for kk in range(KO):
                    t = xs.tile([P, S], BF16, tag="xt")
                    nc.sync.dma_start(t, x_t[kk * P:(kk + 1) * P, :])
                    if extra_dma is not None:
                        extra_dma(kk)
                    for g in range(8):
                        m, n = g // 4, g % 4
                        nc.tensor.matmul(
                            slots[g],
                            w_sb[:, kk, m * P:(m + 1) * P],
                            t[:, n * 512:(n + 1) * 512],
                            start=(kk == 0),
                            stop=(kk == KO - 1),
                        )
                        if kk == KO - 1:
                            # bias-add right behind each group's stop-MM so
                            # psum frees with no end-of-projection bubble
                            nc.vector.tensor_scalar_add(
                                dst_sb[:, m, n * 512:(n + 1) * 512],
                                slots[g], b_sb[:, m:m + 1],
                            )

            def dma_keep(c, sqh):
                nc.sync.dma_start(
                    keep_sb[:, c, sqh * SQC:(sqh + 1) * SQC],
                    keep_t[c * P:(c + 1) * P, sqh * SQC:(sqh + 1) * SQC])

            def proj_v():
                # v[sv, dv] = sum_d xvT[d, sv] * wvT[d, dv], from resident xv
                for w0 in (0, 8):
                    slots = psum_slots()
                    for kk in range(KO):
                        # early mask chunks ride the DMA idle of this phase
                        if kk < 2:
                            dma_keep(w0 // 4 + kk, 0)
                        for g in range(8):
                            sv = w0 + g
                            nc.tensor.matmul(
                                slots[g][:, 0:DQ],
                                xv_sb[:, kk, sv * P:(sv + 1) * P],
                                wv_sb[:, kk, :],
                                start=(kk == 0),
                                stop=(kk == KO - 1),
                            )
                            if kk == KO - 1:
                                nc.vector.tensor_tensor(
                                    v_strided[:, sv, :, 0:64],
                                    slots[g][:, 0:DQ].rearrange(
                                        "p (h c) -> p h c", h=NH),
                                    bv_sb[:].rearrange("p (h c) -> p h c", h=NH),
                                    OP.add,
                                )

            nc.sync.dma_start(wk_sb, wk_t[:].rearrange("p (ko m) -> p ko m", ko=KO))
            nc.sync.dma_start(bk_sb, bk_t[:])
            nc.sync.dma_start(wv_sb, wv_t[:].rearrange("p (ko m) -> p ko m", ko=KO))
            nc.sync.dma_start(bv_sb, bv_t[:])
            # K projection; xv prefetch rides along (resident for V phase)
            proj_qk(xk_t, wk_sb, bk_sb, kT_sb,
                    extra_dma=lambda kk: nc.sync.dma_start(
                        xv_sb[:, kk, :], xv_t[kk * P:(kk + 1) * P, :]))
            nc.sync.dma_start(wq_sb, wq_t[:].rearrange("p (ko m) -> p ko m", ko=KO))
            nc.sync.dma_start(bq_sb, bq_t[:])
            proj_v()
            # Q last: stage B scores can start the moment its biases land
            proj_qk(xq_t, wq_sb, bq_sb, qT_sb)

            # mask chunks (4..15 of sqh0; all of sqh1), then wo
            for c in range(4, SKN):
                dma_keep(c, 0)
            for c in range(SKN):
                dma_keep(c, 1)
            nc.sync.dma_start(wo_sb, wo_t[:].rearrange("p (mq n) -> p mq n", mq=MQ))

            # ---- stage B: attention, ScalarE-saturated per-head sweeps ----
            def normalize(cps, h, sq0):
                """Free cps with one staging copy; normalize lazily.
                HW quirk: custom-DVE / gpsimd ops only work at base partition
                0, so the den row is shifted to partition 0 via SBUF DMA."""
                hb, hm = (h % 2) * 64, h // 2
                ctxu = scp.tile([65, SQC], F32, tag="ctxu")
                nc.vector.tensor_copy(ctxu, cps[0:65, :])
                den0 = scp.tile([1, SQC], F32, tag="den0")
                nc.sync.dma_start(den0, ctxu[64:65, :])
                nc.vector.reciprocal_approx_fast(out=den0, in_=den0)
                scl = scp.tile([64, SQC], F32, tag="scl")
                nc.gpsimd.partition_broadcast(scl, den0[0:1, :])
                cn = scp.tile([64, SQC], BF16, tag="cn")
                nc.vector.tensor_tensor(cn, ctxu[0:64, :], scl, OP.mult)
                nc.sync.dma_start(ctxT_sb[hb:hb + 64, hm, sq0:sq0 + SQC], cn)

            def oproj_unit(do, n2, sq0, eng="v"):
                ps = ps_o.tile([P, 512], F32, tag="o")
                for kk in range(MQ):
                    nc.tensor.matmul(
                        ps,
                        wo_sb[:, kk, do * P:(do + 1) * P],
                        ctxT_sb[:, kk, sq0 + n2 * 512:sq0 + (n2 + 1) * 512],
                        start=(kk == 0),
                        stop=(kk == MQ - 1),
                    )
                ot = outp.tile([P, 512], F16, tag="ot")
                if eng == "v":
                    nc.vector.tensor_copy(ot, ps)
                else:
                    nc.scalar.copy(ot, ps)
                nc.sync.dma_start(
                    out_t[do * P:(do + 1) * P,
                          sq0 + n2 * 512:sq0 + (n2 + 1) * 512], ot)

            def sweep(h, sq0, ounits):
                hb, hm = (h % 2) * 64, h // 2
                cps = ps_c.tile([P, SQC], F32, tag="c")

                def scores(sk):
                    sps = ps_s.tile([P, SQC], F32, tag="s")
                    for j in range(2):
                        nc.tensor.matmul(
                            sps[:, j * 512:(j + 1) * 512],
                            kT_sb[hb:hb + 64, hm, sk * P:(sk + 1) * P],
                            qT_sb[hb:hb + 64, hm,
                                  sq0 + j * 512:sq0 + (j + 1) * 512],
                            start=True, stop=True,
                        )
                    return sps

                def ctx(sk, at):
                    for j in range(2):
                        nc.tensor.matmul(
                            cps[:65, j * 512:(j + 1) * 512],
                            v_sb[:, sk, h * 65:(h + 1) * 65],
                            at[:, j * 512:(j + 1) * 512],
                            start=(sk == 0),
                            stop=(sk == SKN - 1),
                        )

                pend = [scores(0), scores(1)]
                ats = []
                for sk in range(SKN):
                    sps = pend.pop(0)
                    at = attnp.tile([P, SQC], BF16, tag="at")
                    nc.scalar.activation(at, sps, AF.Exp, scale=0.125)
                    nc.vector.tensor_tensor(
                        at, at, keep_sb[:, sk, sq0:sq0 + SQC], OP.mult)
                    if sk + 2 < SKN:
                        pend.append(scores(sk + 2))
                    if ounits and sk >= 10:
                        oproj_unit(*ounits.pop(0))
                    # ctx trails one step so psum hand-off can't block PE
                    ats.append((sk, at))
                    if len(ats) > 1:
                        ctx(*ats.pop(0))
                ctx(*ats.pop(0))
                normalize(cps, h, sq0)

            ounits = []
            for sqh in range(NSQ):
                sq0 = sqh * SQC
                for h in range(NH):
                    sweep(h, sq0, ounits)
                ounits.extend((do, n2, sq0)
                              for do in range(KO) for n2 in range(2))
            for i, u in enumerate(ounits):
                oproj_unit(*u, eng="v" if i % 2 else "s")

    nc.compile()
    return nc


_NC_CACHE = {}


def _get_nc():
    if "nc" not in _NC_CACHE:
        _NC_CACHE["nc"] = build_nc()
    return _NC_CACHE["nc"]


def _pack_w(wT, ko):
    """[D, M] weight (already transposed) -> [P, ko*M] in SBUF layout."""
    d, m = wT.shape
    return np.ascontiguousarray(
        wT.reshape(ko, P, m).transpose(1, 0, 2).reshape(P, ko * m))


def make_in_maps(query, key, value, mask, Wq, bq, Wk, bk, Wv, bv, Wo, bo):
    """Build the 8 per-core input maps (host-side shard + layout prep)."""
    nb = query.shape[0]
    per_b = []
    for b in range(nb):
        xqT = np.ascontiguousarray(query[b].T).astype(NP_BF16)
        xkT = np.ascontiguousarray(key[b].T).astype(NP_BF16)
        xvT = np.ascontiguousarray(value[b].T).astype(NP_BF16)
        keepT = np.ascontiguousarray((~mask[b, 0]).T).astype(NP_BF16)
        per_b.append((xqT, xkT, xvT, keepT))
    per_g = []
    for g in range(4):
        sl = slice(g * DQ, (g + 1) * DQ)
        per_g.append((
            _pack_w(Wq[sl].T.astype(NP_BF16), KO),
            _pack_w(Wk[sl].T.astype(NP_BF16), KO),
            _pack_w(Wv[sl].T.astype(NP_BF16), KO),
            _pack_w(Wo[:, sl].T.astype(NP_BF16), MQ),
            np.ascontiguousarray(bq[sl].reshape(DQ // P, P).T).astype(np.float32),
            np.ascontiguousarray(bk[sl].reshape(DQ // P, P).T).astype(np.float32),
            np.ascontiguousarray(np.broadcast_to(bv[sl], (P, DQ))).astype(np.float32),
        ))
    in_maps = []
    for c in range(NCORES):
        b, g = c // 4, c % 4
        xqT, xkT, xvT, keepT = per_b[b % nb]
        wqT, wkT, wvT, woT, bq2, bk2, bvr = per_g[g]
        in_maps.append({
            "xq": xqT, "xk": xkT, "xv": xvT,
            "wq": wqT, "wk": wkT, "wv": wvT, "wo": woT,
            "bq": bq2, "bk": bk2, "bv": bvr,
            "keep": keepT,
        })
    return in_maps


def gather_output(results, bo, nb=B, s=S, d=D):
    out = np.empty((nb, s, d), np.float32)
    for b in range(nb):
        acc = results[4 * b]["out"].astype(np.float32)
        for g in range(1, 4):
            acc += results[4 * b + g]["out"].astype(np.float32)
        out[b] = acc.T
    out += bo.astype(np.float32)
    return out


def run_on_cores(in_maps, trace=False, **kw):
    nc = _get_nc()
    return run_bass_kernel_spmd(nc, in_maps, list(range(NCORES)), trace=trace, **kw)


def kernel(query, key, value, mask, Wq, bq, Wk, bk, Wv, bv, Wo, bo):
    in_maps = make_in_maps(query, key, value, mask,
                           Wq, bq, Wk, bk, Wv, bv, Wo, bo)
    res = run_on_cores(in_maps, trace=False)
    return gather_output(res.results, bo)


# revision 7
# speedup vs baseline: 1.4767x; 1.0527x over previous
"""Multi-head attention Bass kernel for Trainium2, sharded over 8 NeuronCores.

Sharding: core c handles batch b = c//4 and head-group g = c%4 (4 of 16 heads,
i.e. a 256-wide slice of the QKV projection output).  Each core computes its
heads' attention and a partial output projection (contribution of its 256
ctx columns to the full [S, D] output).  The host sums the 4 partials per
batch (fp32) and adds the output bias.

Device-side design (v4 — ScalarE-saturated pipeline, HAM kept warm):
  - activations shipped pre-transposed: xT = x.T  [D, S]; weights shipped
    pre-packed in the exact SBUF layout (contiguous DMA).
  - scores computed transposed (scoresT[sk, sq]) so attention weights leave
    softmax with sk on partitions — the contraction layout attn@V needs.
  - softmax denominator comes free from a ones-column appended to V
    (ctx psum row 64 = sum_sk attn);  no max-subtraction (scores bounded).
  - stage B processes one head at a time, sweeping sk chunks with a
    double-buffered score psum: the PE issues scores(sk+2) the moment
    exp(sk) frees a buffer, so the Exp stream on ScalarE (the critical
    engine: ~1 elem/cycle/lane) never waits.  ctx matmuls trail one step
    so psum hand-offs can't head-of-line-block the PE queue.
  - HAM (PE clock gate) management: dummy warm-up matmuls cover the DMA
    latency head; the Q projection is split — its hm=1 half is injected
    as filler matmuls into the first sweep so the PE has no idle window
    at the stage A->B transition and stays at 2.4 GHz.
  - normalize frees ctx-psum with one staging copy and runs the
    recip/broadcast/divide lazily; even heads write straight into ctxT.
  - output projection is emitted one unit per sweep step into the PE's
    idle slack during the NEXT sq block's sweeps; final block drains in a
    short tail with psum->SBUF copies split across Vector/Scalar.
  - psum budget (16KB/part): score 2x[128,1024]f32 + ctx 1x[128,1024]f32
    + oproj 2x[128,512]f32 = 8 banks exactly.  Stage A reuses all four
    pools as 8 independent 512-wide accumulation slots, kk-outer.
"""

import numpy as np
import ml_dtypes

import concourse.bass as bass
import concourse.mybir as mybir
import concourse.tile as tile
from concourse import bacc, library_config
from concourse.bass_utils import run_bass_kernel_spmd

# Problem shapes (hardcoded per contest rules).
B, S, D, H, DH = 2, 2048, 1024, 16, 64
NCORES = 8
NH = 4            # heads per core
DQ = NH * DH      # 256: per-core q/k/v width
P = 128

F32 = mybir.dt.float32
F16 = mybir.dt.float16
BF16 = mybir.dt.bfloat16
NP_BF16 = ml_dtypes.bfloat16

SQC = 1024        # sq block per sweep
NSQ = S // SQC    # 2
SKN = S // P      # 16 sk chunks
KO = D // P       # 8 contraction chunks for projections
MQ = DQ // P      # 2


def build_nc():
    """Build the per-core Bass program (same NEFF on all 8 cores)."""
    nc = bacc.Bacc("TRN2", debug=False)

    xq_t = nc.declare_dram_parameter("xq", [D, S], BF16, isOutput=False)
    xk_t = nc.declare_dram_parameter("xk", [D, S], BF16, isOutput=False)
    xv_t = nc.declare_dram_parameter("xv", [D, S], BF16, isOutput=False)
    wq_t = nc.declare_dram_parameter("wq", [P, KO * DQ], BF16, isOutput=False)
    wk_t = nc.declare_dram_parameter("wk", [P, KO * DQ], BF16, isOutput=False)
    wv_t = nc.declare_dram_parameter("wv", [P, KO * DQ], BF16, isOutput=False)
    wo_t = nc.declare_dram_parameter("wo", [P, MQ * D], BF16, isOutput=False)
    bq_t = nc.declare_dram_parameter("bq", [P, MQ], F32, isOutput=False)
    bk_t = nc.declare_dram_parameter("bk", [P, MQ], F32, isOutput=False)
    bv_t = nc.declare_dram_parameter("bv", [P, DQ], F32, isOutput=False)
    keep_t = nc.declare_dram_parameter("keep", [S, S], BF16, isOutput=False)
    out_t = nc.declare_dram_parameter("out", [D, S], F16, isOutput=True)

    AF = mybir.ActivationFunctionType
    OP = mybir.AluOpType

    with tile.TileContext(nc) as tc:
        nc.gpsimd.load_library(library_config.attn)
        with (
            tc.tile_pool(name="const", bufs=1) as const,
            tc.tile_pool(name="xs", bufs=3) as xs,
            tc.tile_pool(name="attn", bufs=7) as attnp,
            tc.tile_pool(name="sc", bufs=2) as scp,
            tc.tile_pool(name="outp", bufs=3) as outp,
            tc.tile_pool(name="ps_s", bufs=2, space="PSUM") as ps_s,
            tc.tile_pool(name="ps_c", bufs=1, space="PSUM") as ps_c,
            tc.tile_pool(name="ps_o", bufs=2, space="PSUM") as ps_o,
        ):
            # ---- persistent SBUF tensors ----
            wq_sb = const.tile([P, KO, DQ], BF16, tag="wq")
            wk_sb = const.tile([P, KO, DQ], BF16, tag="wk")
            wv_sb = const.tile([P, KO, DQ], BF16, tag="wv")
            wo_sb = const.tile([P, MQ, D], BF16, tag="wo")
            bq_sb = const.tile([P, MQ], F32, tag="bq")
            bk_sb = const.tile([P, MQ], F32, tag="bk")
            bv_sb = const.tile([P, DQ], F32, tag="bv")
            qT_sb = const.tile([P, MQ, S], BF16, tag="qT")
            kT_sb = const.tile([P, MQ, S], BF16, tag="kT")
            v_sb = const.tile([P, SKN, NH * 65], BF16, tag="v")
            keep_sb = const.tile([P, SKN, S], BF16, tag="keep")
            ctxT_sb = const.tile([P, MQ, S], BF16, tag="ctxT")
            xa_sb = const.tile([P, KO, S], BF16, tag="xa")  # resident xv->xq
            warm = const.tile([1, 8], F32, tag="warm")
            wmm = const.tile([1, 512], BF16, tag="wmm")

            # preload the exp table set on ScalarE while stage A runs
            nc.vector.memset(warm, 0.0)
            nc.scalar.activation(warm, warm, AF.Exp, scale=1.0)
            nc.vector.memset(wmm, 0.0)

            pswarm = ps_o.tile([P, 512], F32, tag="o")

            def warm_burst(lhs, rhs, n=10):
                """Dummy matmuls to keep the PE HAM clock-gate open."""
                w = rhs.shape[-1]
                for _ in range(n):
                    nc.tensor.matmul(pswarm[0:64, 0:w], lhs,
                                     rhs, start=True, stop=True)

            warm_burst(wmm[0:1, 0:64], wmm[0:1, 0:512], 12)

            # ones column per head in the V tile (softmax denominator trick)
            v_strided = v_sb[:].rearrange("p s (h c) -> p s h c", h=NH)
            nc.vector.memset(v_strided[:, :, :, 64:65], 1.0)

            def psum_slots():
                """8 independent [128,512] accumulation slots spanning all
                psum pools (stage A only; stage B owns the pools then)."""
                t0 = ps_s.tile([P, SQC], F32, tag="s")
                t1 = ps_s.tile([P, SQC], F32, tag="s")
                tcx = ps_c.tile([P, SQC], F32, tag="c")
                o0 = ps_o.tile([P, 512], F32, tag="o")
                o1 = ps_o.tile([P, 512], F32, tag="o")
                return [t0[:, 0:512], t0[:, 512:1024],
                        t1[:, 0:512], t1[:, 512:1024],
                        tcx[:, 0:512], tcx[:, 512:1024],
                        o0[:, :], o1[:, :]]

            def dma_keep(c, sqh):
                nc.sync.dma_start(
                    keep_sb[:, c, sqh * SQC:(sqh + 1) * SQC],
                    keep_t[c * P:(c + 1) * P, sqh * SQC:(sqh + 1) * SQC])

            # ---- stage A ----
            # K projection: stream xk, prefetch xv halves into resident xa
            nc.sync.dma_start(wk_sb, wk_t[:].rearrange("p (ko m) -> p ko m", ko=KO))
            nc.sync.dma_start(bk_sb, bk_t[:])
            # dep on wk DMA: covers the DMA-latency head before K's matmuls
            warm_burst(wk_sb[0:1, 0, 0:64], wk_sb[0:1, 0, 0:256], 24)

            slots = psum_slots()
            for kk in range(KO):
                t = xs.tile([P, S], BF16, tag="xt")
                nc.sync.dma_start(t, xk_t[kk * P:(kk + 1) * P, :])
                nc.sync.dma_start(xa_sb[:, kk, 0:SQC],
                                  xv_t[kk * P:(kk + 1) * P, 0:SQC])
                for g in range(8):
                    m, n = g // 4, g % 4
                    nc.tensor.matmul(
                        slots[g],
                        wk_sb[:, kk, m * P:(m + 1) * P],
                        t[:, n * 512:(n + 1) * 512],
                        start=(kk == 0), stop=(kk == KO - 1),
                    )
                    if kk == KO - 1:
                        nc.vector.tensor_scalar_add(
                            kT_sb[:, m, n * 512:(n + 1) * 512],
                            slots[g], bk_sb[:, m:m + 1])

            nc.sync.dma_start(wv_sb, wv_t[:].rearrange("p (ko m) -> p ko m", ko=KO))
            nc.sync.dma_start(bv_sb, bv_t[:])
            nc.sync.dma_start(wq_sb, wq_t[:].rearrange("p (ko m) -> p ko m", ko=KO))
            nc.sync.dma_start(bq_sb, bq_t[:])

            # V projection from resident xa; second xv half + xq ride along
            for w0 in (0, 8):
                slots = psum_slots()
                for kk in range(KO):
                    if w0 == 0:
                        nc.sync.dma_start(xa_sb[:, kk, SQC:S],
                                          xv_t[kk * P:(kk + 1) * P, SQC:S])
                        if kk < 4:
                            dma_keep(kk, 0)
                    for g in range(8):
                        sv = w0 + g
                        nc.tensor.matmul(
                            slots[g][:, 0:DQ],
                            xa_sb[:, kk, sv * P:(sv + 1) * P],
                            wv_sb[:, kk, :],
                            start=(kk == 0), stop=(kk == KO - 1),
                        )
                        if kk == KO - 1:
                            nc.vector.tensor_tensor(
                                v_strided[:, sv, :, 0:64],
                                slots[g][:, 0:DQ].rearrange(
                                    "p (h c) -> p h c", h=NH),
                                bv_sb[:].rearrange("p (h c) -> p h c", h=NH),
                                OP.add,
                            )
                    if w0 == 8:
                        # xq overwrites xa[kk] once wave1 has read it
                        nc.sync.dma_start(xa_sb[:, kk, :],
                                          xq_t[kk * P:(kk + 1) * P, :])
                        if kk < 4:
                            dma_keep(4 + kk, 0)

            # Q projection, hm=0 half (heads 0,1) — stage B starts after this
            t0 = ps_s.tile([P, SQC], F32, tag="s")
            t1 = ps_s.tile([P, SQC], F32, tag="s")
            qslots = [t0[:, 0:512], t0[:, 512:1024],
                      t1[:, 0:512], t1[:, 512:1024]]
            for kk in range(KO):
                for n in range(4):
                    nc.tensor.matmul(
                        qslots[n],
                        wq_sb[:, kk, 0:P],
                        xa_sb[:, kk, n * 512:(n + 1) * 512],
                        start=(kk == 0), stop=(kk == KO - 1),
                    )
                    if kk == KO - 1:
                        nc.vector.tensor_scalar_add(
                            qT_sb[:, 0, n * 512:(n + 1) * 512],
                            qslots[n], bq_sb[:, 0:1])

            # remaining mask chunks + wo (ride DMA during early stage B)
            for c in range(8, SKN):
                dma_keep(c, 0)
            for c in range(SKN):
                dma_keep(c, 1)
            nc.sync.dma_start(wo_sb, wo_t[:].rearrange("p (mq n) -> p mq n", mq=MQ))

            # Q hm=1 half as filler closures, injected into sweep(h0)
            qm1_state = {}

            def qm1_filler(sub, kk):
                if kk == 0:
                    qm1_state[sub] = [
                        ps_o.tile([P, 512], F32, name=f"qm1_{sub}_{i}", tag="o")
                        for i in range(2)]
                tiles = qm1_state[sub]
                for i in range(2):
                    n = sub * 2 + i
                    nc.tensor.matmul(
                        tiles[i],
                        wq_sb[:, kk, P:2 * P],
                        xa_sb[:, kk, n * 512:(n + 1) * 512],
                        start=(kk == 0), stop=(kk == KO - 1),
                    )
                    if kk == KO - 1:
                        nc.vector.tensor_scalar_add(
                            qT_sb[:, 1, n * 512:(n + 1) * 512],
                            tiles[i], bq_sb[:, 1:2])

            fillers = [lambda sub=sub, kk=kk: qm1_filler(sub, kk)
                       for sub in range(2) for kk in range(KO)]

            # ---- stage B: attention, ScalarE-saturated per-head sweeps ----
            def normalize(cps, h, sq0):
                """Free cps with one staging copy; normalize lazily.
                HW quirk: custom-DVE / gpsimd ops only work at base partition
                0, so the den row is shifted to partition 0 via SBUF DMA."""
                hb, hm = (h % 2) * 64, h // 2
                ctxu = scp.tile([65, SQC], F32, tag="ctxu")
                nc.vector.tensor_copy(ctxu, cps[0:65, :])
                den0 = scp.tile([1, SQC], F32, tag="den0")
                nc.sync.dma_start(den0, ctxu[64:65, :])
                nc.vector.reciprocal_approx_fast(out=den0, in_=den0)
                scl = scp.tile([64, SQC], F32, tag="scl")
                nc.gpsimd.partition_broadcast(scl, den0[0:1, :])
                if hb == 0:
                    nc.vector.tensor_tensor(
                        ctxT_sb[0:64, hm, sq0:sq0 + SQC],
                        ctxu[0:64, :], scl, OP.mult)
                else:
                    cn = scp.tile([64, SQC], BF16, tag="cn")
                    nc.vector.tensor_tensor(cn, ctxu[0:64, :], scl, OP.mult)
                    nc.sync.dma_start(
                        ctxT_sb[64:128, hm, sq0:sq0 + SQC], cn)

            def oproj_unit(do, n2, sq0, eng="v"):
                ps = ps_o.tile([P, 512], F32, tag="o")
                for kk in range(MQ):
                    nc.tensor.matmul(
                        ps,
                        wo_sb[:, kk, do * P:(do + 1) * P],
                        ctxT_sb[:, kk, sq0 + n2 * 512:sq0 + (n2 + 1) * 512],
                        start=(kk == 0), stop=(kk == MQ - 1),
                    )
                ot = outp.tile([P, 512], F16, tag="ot")
                if eng == "v":
                    nc.vector.tensor_copy(ot, ps)
                else:
                    nc.scalar.copy(ot, ps)
                nc.sync.dma_start(
                    out_t[do * P:(do + 1) * P,
                          sq0 + n2 * 512:sq0 + (n2 + 1) * 512], ot)

            def sweep(h, sq0, ounits, fill):
                hb, hm = (h % 2) * 64, h // 2
                cps = ps_c.tile([P, SQC], F32, tag="c")

                def scores(sk):
                    sps = ps_s.tile([P, SQC], F32, tag="s")
                    for j in range(2):
                        nc.tensor.matmul(
                            sps[:, j * 512:(j + 1) * 512],
                            kT_sb[hb:hb + 64, hm, sk * P:(sk + 1) * P],
                            qT_sb[hb:hb + 64, hm,
                                  sq0 + j * 512:sq0 + (j + 1) * 512],
                            start=True, stop=True,
                        )
                    return sps

                def ctx(sk, at):
                    for j in range(2):
                        nc.tensor.matmul(
                            cps[:65, j * 512:(j + 1) * 512],
                            v_sb[:, sk, h * 65:(h + 1) * 65],
                            at[:, j * 512:(j + 1) * 512],
                            start=(sk == 0),
                            stop=(sk == SKN - 1),
                        )

                pend = [scores(0), scores(1)]
                ats = []
                for sk in range(SKN):
                    sps = pend.pop(0)
                    at = attnp.tile([P, SQC], BF16, tag="at")
                    nc.scalar.activation(at, sps, AF.Exp, scale=0.125)
                    nc.vector.tensor_tensor(
                        at, at, keep_sb[:, sk, sq0:sq0 + SQC], OP.mult)
                    # PE fillers ride the ramp / idle slack
                    for _ in range(2 if sk < 3 else 1):
                        if fill:
                            fill.pop(0)()
                    if sk + 2 < SKN:
                        pend.append(scores(sk + 2))
                    if ounits and sk >= 10:
                        oproj_unit(*ounits.pop(0))
                    # ctx trails one step so psum hand-off can't block PE
                    ats.append((sk, at))
                    if len(ats) > 1:
                        ctx(*ats.pop(0))
                ctx(*ats.pop(0))
                normalize(cps, h, sq0)

            ounits = []
            for sqh in range(NSQ):
                sq0 = sqh * SQC
                for h in range(NH):
                    sweep(h, sq0, ounits,
                          fillers if (sqh == 0 and h == 0) else None)
                ounits.extend((do, n2, sq0)
                              for do in range(KO) for n2 in range(2))
            for i, u in enumerate(ounits):
                oproj_unit(*u, eng="v" if i % 2 else "s")

    nc.compile()
    return nc


_NC_CACHE = {}


def _get_nc():
    if "nc" not in _NC_CACHE:
        _NC_CACHE["nc"] = build_nc()
    return _NC_CACHE["nc"]


def _pack_w(wT, ko):
    """[D, M] weight (already transposed) -> [P, ko*M] in SBUF layout."""
    d, m = wT.shape
    return np.ascontiguousarray(
        wT.reshape(ko, P, m).transpose(1, 0, 2).reshape(P, ko * m))


def make_in_maps(query, key, value, mask, Wq, bq, Wk, bk, Wv, bv, Wo, bo):
    """Build the 8 per-core input maps (host-side shard + layout prep)."""
    nb = query.shape[0]
    per_b = []
    for b in range(nb):
        xqT = np.ascontiguousarray(query[b].T).astype(NP_BF16)
        xkT = np.ascontiguousarray(key[b].T).astype(NP_BF16)
        xvT = np.ascontiguousarray(value[b].T).astype(NP_BF16)
        keepT = np.ascontiguousarray((~mask[b, 0]).T).astype(NP_BF16)
        per_b.append((xqT, xkT, xvT, keepT))
    per_g = []
    for g in range(4):
        sl = slice(g * DQ, (g + 1) * DQ)
        per_g.append((
            _pack_w(Wq[sl].T.astype(NP_BF16), KO),
            _pack_w(Wk[sl].T.astype(NP_BF16), KO),
            _pack_w(Wv[sl].T.astype(NP_BF16), KO),
            _pack_w(Wo[:, sl].T.astype(NP_BF16), MQ),
            np.ascontiguousarray(bq[sl].reshape(DQ // P, P).T).astype(np.float32),
            np.ascontiguousarray(bk[sl].reshape(DQ // P, P).T).astype(np.float32),
            np.ascontiguousarray(np.broadcast_to(bv[sl], (P, DQ))).astype(np.float32),
        ))
    in_maps = []
    for c in range(NCORES):
        b, g = c // 4, c % 4
        xqT, xkT, xvT, keepT = per_b[b % nb]
        wqT, wkT, wvT, woT, bq2, bk2, bvr = per_g[g]
        in_maps.append({
            "xq": xqT, "xk": xkT, "xv": xvT,
            "wq": wqT, "wk": wkT, "wv": wvT, "wo": woT,
            "bq": bq2, "bk": bk2, "bv": bvr,
            "keep": keepT,
        })
    return in_maps


def gather_output(results, bo, nb=B, s=S, d=D):
    out = np.empty((nb, s, d), np.float32)
    for b in range(nb):
        acc = results[4 * b]["out"].astype(np.float32)
        for g in range(1, 4):
            acc += results[4 * b + g]["out"].astype(np.float32)
        out[b] = acc.T
    out += bo.astype(np.float32)
    return out


def run_on_cores(in_maps, trace=False, **kw):
    nc = _get_nc()
    return run_bass_kernel_spmd(nc, in_maps, list(range(NCORES)), trace=trace, **kw)


def kernel(query, key, value, mask, Wq, bq, Wk, bk, Wv, bv, Wo, bo):
    in_maps = make_in_maps(query, key, value, mask,
                           Wq, bq, Wk, bk, Wv, bv, Wo, bo)
    res = run_on_cores(in_maps, trace=False)
    return gather_output(res.results, bo)


# revision 13
# speedup vs baseline: 1.5595x; 1.0561x over previous
"""Multi-head attention Bass kernel for Trainium2, sharded over 8 NeuronCores.

Sharding: core c handles batch b = c//4 and head-group g = c%4 (4 of 16 heads,
i.e. a 256-wide slice of the QKV projection output).  Each core computes its
heads' attention and a partial output projection (contribution of its 256
ctx columns to the full [S, D] output).  The host sums the 4 partials per
batch (fp32) and adds the output bias.

Device-side design (v4 — ScalarE-saturated pipeline, HAM kept warm):
  - activations shipped pre-transposed: xT = x.T  [D, S]; weights shipped
    pre-packed in the exact SBUF layout (contiguous DMA).
  - scores computed transposed (scoresT[sk, sq]) so attention weights leave
    softmax with sk on partitions — the contraction layout attn@V needs.
  - softmax denominator comes free from a ones-column appended to V
    (ctx psum row 64 = sum_sk attn);  no max-subtraction (scores bounded).
  - stage B processes one head at a time, sweeping sk chunks with a
    double-buffered score psum: the PE issues scores(sk+2) the moment
    exp(sk) frees a buffer, so the Exp stream on ScalarE (the critical
    engine: ~1 elem/cycle/lane) never waits.  ctx matmuls trail one step
    so psum hand-offs can't head-of-line-block the PE queue.
  - HAM (PE clock gate) management: dummy warm-up matmuls cover the DMA
    latency head; the Q projection is split — its hm=1 half is injected
    as filler matmuls into the first sweep so the PE has no idle window
    at the stage A->B transition and stays at 2.4 GHz.
  - normalize frees ctx-psum with one staging copy and runs the
    recip/broadcast/divide lazily; even heads write straight into ctxT.
  - output projection is emitted one unit per sweep step into the PE's
    idle slack during the NEXT sq block's sweeps; final block drains in a
    short tail with psum->SBUF copies split across Vector/Scalar.
  - psum budget (16KB/part): score 2x[128,1024]f32 + ctx 1x[128,1024]f32
    + oproj 2x[128,512]f32 = 8 banks exactly.  Stage A reuses all four
    pools as 8 independent 512-wide accumulation slots, kk-outer.
"""

import numpy as np
import ml_dtypes

import concourse.bass as bass
import concourse.mybir as mybir
import concourse.tile as tile
from concourse import bacc, library_config
from concourse.bass_utils import run_bass_kernel_spmd

# Problem shapes (hardcoded per contest rules).
B, S, D, H, DH = 2, 2048, 1024, 16, 64
NCORES = 8
NH = 4            # heads per core
DQ = NH * DH      # 256: per-core q/k/v width
P = 128

F32 = mybir.dt.float32
F16 = mybir.dt.float16
BF16 = mybir.dt.bfloat16
NP_BF16 = ml_dtypes.bfloat16

SQC = 1024        # sq block per sweep
NSQ = S // SQC    # 2
SKN = S // P      # 16 sk chunks
KO = D // P       # 8 contraction chunks for projections
MQ = DQ // P      # 2


def build_nc():
    """Build the per-core Bass program (same NEFF on all 8 cores)."""
    nc = bacc.Bacc("TRN2", debug=False)

    xq_t = nc.declare_dram_parameter("xq", [D, S], BF16, isOutput=False)
    xk_t = nc.declare_dram_parameter("xk", [D, S], BF16, isOutput=False)
    xv_t = nc.declare_dram_parameter("xv", [D, S], BF16, isOutput=False)
    wq_t = nc.declare_dram_parameter("wq", [P, KO * DQ], BF16, isOutput=False)
    wk_t = nc.declare_dram_parameter("wk", [P, KO * DQ], BF16, isOutput=False)
    wv_t = nc.declare_dram_parameter("wv", [P, KO * DQ], BF16, isOutput=False)
    wo_t = nc.declare_dram_parameter("wo", [P, MQ * D], BF16, isOutput=False)
    bq_t = nc.declare_dram_parameter("bq", [P, MQ], F32, isOutput=False)
    bk_t = nc.declare_dram_parameter("bk", [P, MQ], F32, isOutput=False)
    bv_t = nc.declare_dram_parameter("bv", [P, DQ], F32, isOutput=False)
    keep_t = nc.declare_dram_parameter("keep", [S, S], BF16, isOutput=False)
    out_t = nc.declare_dram_parameter("out", [D, S], F16, isOutput=True)

    AF = mybir.ActivationFunctionType
    OP = mybir.AluOpType

    with tile.TileContext(nc) as tc:
        nc.gpsimd.load_library(library_config.attn)
        with (
            tc.tile_pool(name="const", bufs=1) as const,
            tc.tile_pool(name="xs", bufs=3) as xs,
            tc.tile_pool(name="attn", bufs=7) as attnp,
            tc.tile_pool(name="sc", bufs=2) as scp,
            tc.tile_pool(name="outp", bufs=3) as outp,
            tc.tile_pool(name="ps_s", bufs=2, space="PSUM") as ps_s,
            tc.tile_pool(name="ps_c", bufs=1, space="PSUM") as ps_c,
            tc.tile_pool(name="ps_o", bufs=2, space="PSUM") as ps_o,
        ):
            # ---- persistent SBUF tensors ----
            wq_sb = const.tile([P, KO, DQ], BF16, tag="wq")
            wk_sb = const.tile([P, KO, DQ], BF16, tag="wk")
            wv_sb = const.tile([P, KO, DQ], BF16, tag="wv")
            wo_sb = const.tile([P, MQ, D], BF16, tag="wo")
            bq_sb = const.tile([P, MQ], F32, tag="bq")
            bk_sb = const.tile([P, MQ], F32, tag="bk")
            bv_sb = const.tile([P, DQ], F32, tag="bv")
            qT_sb = const.tile([P, MQ, S], BF16, tag="qT")
            kT_sb = const.tile([P, MQ, S], BF16, tag="kT")
            v_sb = const.tile([P, SKN, NH * 65], BF16, tag="v")
            keep_sb = const.tile([P, SKN, S], BF16, tag="keep")
            ctxT_sb = const.tile([P, MQ, S], BF16, tag="ctxT")
            xa_sb = const.tile([P, KO, S], BF16, tag="xa")  # resident xv->xq
            warm = const.tile([1, 8], F32, tag="warm")
            wmm = const.tile([1, 512], BF16, tag="wmm")

            # preload the exp table set on ScalarE while stage A runs
            nc.vector.memset(warm, 0.0)
            nc.scalar.activation(warm, warm, AF.Exp, scale=1.0)
            nc.vector.memset(wmm, 0.0)

            pswarm = ps_o.tile([P, 512], F32, tag="o")

            def warm_burst(lhs, rhs, n=10):
                """Dummy matmuls to keep the PE HAM clock-gate open."""
                w = rhs.shape[-1]
                for _ in range(n):
                    nc.tensor.matmul(pswarm[0:64, 0:w], lhs,
                                     rhs, start=True, stop=True)

            warm_burst(wmm[0:1, 0:64], wmm[0:1, 0:512], 12)

            # ones column per head in the V tile (softmax denominator trick)
            v_strided = v_sb[:].rearrange("p s (h c) -> p s h c", h=NH)
            nc.vector.memset(v_strided[:, :, :, 64:65], 1.0)

            def psum_slots():
                """8 independent [128,512] accumulation slots spanning all
                psum pools (stage A only; stage B owns the pools then)."""
                t0 = ps_s.tile([P, SQC], F32, tag="s")
                t1 = ps_s.tile([P, SQC], F32, tag="s")
                tcx = ps_c.tile([P, SQC], F32, tag="c")
                o0 = ps_o.tile([P, 512], F32, tag="o")
                o1 = ps_o.tile([P, 512], F32, tag="o")
                return [t0[:, 0:512], t0[:, 512:1024],
                        t1[:, 0:512], t1[:, 512:1024],
                        tcx[:, 0:512], tcx[:, 512:1024],
                        o0[:, :], o1[:, :]]

            def dma_keep(c, sqh):
                nc.sync.dma_start(
                    keep_sb[:, c, sqh * SQC:(sqh + 1) * SQC],
                    keep_t[c * P:(c + 1) * P, sqh * SQC:(sqh + 1) * SQC])

            # ---- stage A ----
            # K projection: stream xk, prefetch xv halves into resident xa
            nc.sync.dma_start(wk_sb, wk_t[:].rearrange("p (ko m) -> p ko m", ko=KO))
            nc.sync.dma_start(bk_sb, bk_t[:])
            # dep on wk DMA: covers the DMA-latency head before K's matmuls
            warm_burst(wk_sb[0:1, 0, 0:64], wk_sb[0:1, 0, 0:256], 12)

            slots = psum_slots()
            for kk in range(KO):
                t = xs.tile([P, S], BF16, tag="xt")
                nc.sync.dma_start(t, xk_t[kk * P:(kk + 1) * P, :])
                for g in range(8):
                    m, n = g // 4, g % 4
                    nc.tensor.matmul(
                        slots[g],
                        wk_sb[:, kk, m * P:(m + 1) * P],
                        t[:, n * 512:(n + 1) * 512],
                        start=(kk == 0), stop=(kk == KO - 1),
                    )
                    if kk == KO - 1:
                        nc.vector.tensor_scalar_add(
                            kT_sb[:, m, n * 512:(n + 1) * 512],
                            slots[g], bk_sb[:, m:m + 1])

            nc.sync.dma_start(wv_sb, wv_t[:].rearrange("p (ko m) -> p ko m", ko=KO))
            nc.sync.dma_start(bv_sb, bv_t[:])
            # xv first halves stream just-in-time into wave0
            for kk in range(KO):
                nc.sync.dma_start(xa_sb[:, kk, 0:SQC],
                                  xv_t[kk * P:(kk + 1) * P, 0:SQC])
            nc.sync.dma_start(wq_sb, wq_t[:].rearrange("p (ko m) -> p ko m", ko=KO))
            nc.sync.dma_start(bq_sb, bq_t[:])

            # V projection from resident xa; second xv half + xq ride along
            for w0 in (0, 8):
                slots = psum_slots()
                for kk in range(KO):
                    if w0 == 0:
                        nc.sync.dma_start(xa_sb[:, kk, SQC:S],
                                          xv_t[kk * P:(kk + 1) * P, SQC:S])
                    for g in range(8):
                        sv = w0 + g
                        nc.tensor.matmul(
                            slots[g][:, 0:DQ],
                            xa_sb[:, kk, sv * P:(sv + 1) * P],
                            wv_sb[:, kk, :],
                            start=(kk == 0), stop=(kk == KO - 1),
                        )
                        if kk == KO - 1:
                            nc.vector.tensor_tensor(
                                v_strided[:, sv, :, 0:64],
                                slots[g][:, 0:DQ].rearrange(
                                    "p (h c) -> p h c", h=NH),
                                bv_sb[:].rearrange("p (h c) -> p h c", h=NH),
                                OP.add,
                            )
                    if w0 == 8:
                        # xq overwrites xa[kk] once wave1 has read it
                        nc.sync.dma_start(xa_sb[:, kk, :],
                                          xq_t[kk * P:(kk + 1) * P, :])

            # Q projection, hm=0 half (heads 0,1) — stage B starts after this
            t0 = ps_s.tile([P, SQC], F32, tag="s")
            t1 = ps_s.tile([P, SQC], F32, tag="s")
            qslots = [t0[:, 0:512], t0[:, 512:1024],
                      t1[:, 0:512], t1[:, 512:1024]]
            for kk in range(KO):
                for n in range(4):
                    nc.tensor.matmul(
                        qslots[n],
                        wq_sb[:, kk, 0:P],
                        xa_sb[:, kk, n * 512:(n + 1) * 512],
                        start=(kk == 0), stop=(kk == KO - 1),
                    )
                    if kk == KO - 1:
                        nc.vector.tensor_scalar_add(
                            qT_sb[:, 0, n * 512:(n + 1) * 512],
                            qslots[n], bq_sb[:, 0:1])

            # mask chunks + wo (ride DMA during early stage B, after xq)
            for c in range(SKN):
                dma_keep(c, 0)
            nc.sync.dma_start(wo_sb, wo_t[:].rearrange("p (mq n) -> p mq n", mq=MQ))
            for c in range(SKN):
                dma_keep(c, 1)

            # Q hm=1 half as filler closures, injected into sweep(h0)
            qm1_state = {}

            def qm1_filler(sub, kk):
                if kk == 0:
                    qm1_state[sub] = [
                        ps_o.tile([P, 512], F32, name=f"qm1_{sub}_{i}", tag="o")
                        for i in range(2)]
                tiles = qm1_state[sub]
                for i in range(2):
                    n = sub * 2 + i
                    nc.tensor.matmul(
                        tiles[i],
                        wq_sb[:, kk, P:2 * P],
                        xa_sb[:, kk, n * 512:(n + 1) * 512],
                        start=(kk == 0), stop=(kk == KO - 1),
                    )
                    if kk == KO - 1:
                        nc.vector.tensor_scalar_add(
                            qT_sb[:, 1, n * 512:(n + 1) * 512],
                            tiles[i], bq_sb[:, 1:2])

            fillers = [lambda sub=sub, kk=kk: qm1_filler(sub, kk)
                       for sub in range(2) for kk in range(KO)]

            # ---- stage B: attention, ScalarE-saturated per-head sweeps ----
            def normalize(cps, h, sq0):
                """Free cps with one staging copy; normalize lazily.
                HW quirk: custom-DVE / gpsimd ops only work at base partition
                0, so the den row is shifted to partition 0 via SBUF DMA."""
                hb, hm = (h % 2) * 64, h // 2
                ctxu = scp.tile([65, SQC], F32, tag="ctxu")
                nc.vector.tensor_copy(ctxu, cps[0:65, :])
                den0 = scp.tile([1, SQC], F32, tag="den0")
                nc.sync.dma_start(den0, ctxu[64:65, :])
                nc.vector.reciprocal_approx_fast(out=den0, in_=den0)
                scl = scp.tile([64, SQC], F32, tag="scl")
                nc.gpsimd.partition_broadcast(scl, den0[0:1, :])
                if hb == 0:
                    nc.vector.tensor_tensor(
                        ctxT_sb[0:64, hm, sq0:sq0 + SQC],
                        ctxu[0:64, :], scl, OP.mult)
                else:
                    cn = scp.tile([64, SQC], BF16, tag="cn")
                    nc.vector.tensor_tensor(cn, ctxu[0:64, :], scl, OP.mult)
                    nc.sync.dma_start(
                        ctxT_sb[64:128, hm, sq0:sq0 + SQC], cn)

            def oproj_unit(do, n2, sq0, eng="v"):
                ps = ps_o.tile([P, 512], F32, tag="o")
                for kk in range(MQ):
                    nc.tensor.matmul(
                        ps,
                        wo_sb[:, kk, do * P:(do + 1) * P],
                        ctxT_sb[:, kk, sq0 + n2 * 512:sq0 + (n2 + 1) * 512],
                        start=(kk == 0), stop=(kk == MQ - 1),
                    )
                ot = outp.tile([P, 512], F16, tag="ot")
                if eng == "v":
                    nc.vector.tensor_copy(ot, ps)
                else:
                    nc.scalar.copy(ot, ps)
                nc.sync.dma_start(
                    out_t[do * P:(do + 1) * P,
                          sq0 + n2 * 512:sq0 + (n2 + 1) * 512], ot)

            def sweep(h, sq0, ounits, fill):
                hb, hm = (h % 2) * 64, h // 2
                cps = ps_c.tile([P, SQC], F32, tag="c")

                def scores(sk):
                    sps = ps_s.tile([P, SQC], F32, tag="s")
                    for j in range(2):
                        nc.tensor.matmul(
                            sps[:, j * 512:(j + 1) * 512],
                            kT_sb[hb:hb + 64, hm, sk * P:(sk + 1) * P],
                            qT_sb[hb:hb + 64, hm,
                                  sq0 + j * 512:sq0 + (j + 1) * 512],
                            start=True, stop=True,
                        )
                    return sps

                def ctx(sk, at):
                    for j in range(2):
                        nc.tensor.matmul(
                            cps[:65, j * 512:(j + 1) * 512],
                            v_sb[:, sk, h * 65:(h + 1) * 65],
                            at[:, j * 512:(j + 1) * 512],
                            start=(sk == 0),
                            stop=(sk == SKN - 1),
                        )

                pend = [scores(0), scores(1)]
                ats = []
                for sk in range(SKN):
                    sps = pend.pop(0)
                    at = attnp.tile([P, SQC], BF16, tag="at")
                    nc.scalar.activation(at, sps, AF.Exp, scale=0.125)
                    nc.vector.tensor_tensor(
                        at, at, keep_sb[:, sk, sq0:sq0 + SQC], OP.mult)
                    # PE fillers ride the ramp / idle slack
                    for _ in range(2 if sk < 3 else 1):
                        if fill:
                            fill.pop(0)()
                    if sk + 2 < SKN:
                        pend.append(scores(sk + 2))
                    if ounits and sk >= 8 and sk % 2 == 0:
                        oproj_unit(*ounits.pop(0))
                    # ctx trails one step so psum hand-off can't block PE
                    ats.append((sk, at))
                    if len(ats) > 1:
                        ctx(*ats.pop(0))
                ctx(*ats.pop(0))
                normalize(cps, h, sq0)

            ounits = []
            for sqh in range(NSQ):
                sq0 = sqh * SQC
                # odd heads first: the tail-gating last sweeps then write
                # their normalized ctx straight into ctxT (no shift DMA)
                for i, h in enumerate((1, 3, 0, 2)):
                    sweep(h, sq0, ounits,
                          fillers if (sqh == 0 and i == 0) else None)
                ounits.extend((do, n2, sq0)
                              for do in range(KO) for n2 in range(2))
            for i, u in enumerate(ounits):
                oproj_unit(*u, eng="v" if i % 2 else "s")

    nc.compile()
    return nc


_NC_CACHE = {}


def _get_nc():
    if "nc" not in _NC_CACHE:
        _NC_CACHE["nc"] = build_nc()
    return _NC_CACHE["nc"]


def _pack_w(wT, ko):
    """[D, M] weight (already transposed) -> [P, ko*M] in SBUF layout."""
    d, m = wT.shape
    return np.ascontiguousarray(
        wT.reshape(ko, P, m).transpose(1, 0, 2).reshape(P, ko * m))


def make_in_maps(query, key, value, mask, Wq, bq, Wk, bk, Wv, bv, Wo, bo):
    """Build the 8 per-core input maps (host-side shard + layout prep)."""
    nb = query.shape[0]
    per_b = []
    for b in range(nb):
        xqT = np.ascontiguousarray(query[b].T).astype(NP_BF16)
        xkT = np.ascontiguousarray(key[b].T).astype(NP_BF16)
        xvT = np.ascontiguousarray(value[b].T).astype(NP_BF16)
        keepT = np.ascontiguousarray((~mask[b, 0]).T).astype(NP_BF16)
        per_b.append((xqT, xkT, xvT, keepT))
    per_g = []
    for g in range(4):
        sl = slice(g * DQ, (g + 1) * DQ)
        per_g.append((
            _pack_w(Wq[sl].T.astype(NP_BF16), KO),
            _pack_w(Wk[sl].T.astype(NP_BF16), KO),
            _pack_w(Wv[sl].T.astype(NP_BF16), KO),
            _pack_w(Wo[:, sl].T.astype(NP_BF16), MQ),
            np.ascontiguousarray(bq[sl].reshape(DQ // P, P).T).astype(np.float32),
            np.ascontiguousarray(bk[sl].reshape(DQ // P, P).T).astype(np.float32),
            np.ascontiguousarray(np.broadcast_to(bv[sl], (P, DQ))).astype(np.float32),
        ))
    in_maps = []
    for c in range(NCORES):
        b, g = c // 4, c % 4
        xqT, xkT, xvT, keepT = per_b[b % nb]
        wqT, wkT, wvT, woT, bq2, bk2, bvr = per_g[g]
        in_maps.append({
            "xq": xqT, "xk": xkT, "xv": xvT,
            "wq": wqT, "wk": wkT, "wv": wvT, "wo": woT,
            "bq": bq2, "bk": bk2, "bv": bvr,
            "keep": keepT,
        })
    return in_maps


def gather_output(results, bo, nb=B, s=S, d=D):
    out = np.empty((nb, s, d), np.float32)
    for b in range(nb):
        acc = results[4 * b]["out"].astype(np.float32)
        for g in range(1, 4):
            acc += results[4 * b + g]["out"].astype(np.float32)
        out[b] = acc.T
    out += bo.astype(np.float32)
    return out


def run_on_cores(in_maps, trace=False, **kw):
    nc = _get_nc()
    return run_bass_kernel_spmd(nc, in_maps, list(range(NCORES)), trace=trace, **kw)


def kernel(query, key, value, mask, Wq, bq, Wk, bk, Wv, bv, Wo, bo):
    in_maps = make_in_maps(query, key, value, mask,
                           Wq, bq, Wk, bk, Wv, bv, Wo, bo)
    res = run_on_cores(in_maps, trace=False)
    return gather_output(res.results, bo)
